# revision 25
# baseline (speedup 1.0000x reference)
"""Clifford algebra geometric product kernel for 8 Trainium2 NeuronCores.

out[..., j] = sum_{i,k} a[..., i] * cayley[i, j, k] * b[..., k]
with cayley the Cl(3,0) (metric [1,1,1]) geometric-product table in
short-lex blade order.  a, b: [65536, 64, 8] float32.

Sharding: pure data parallel over the leading batch axis (8192 batches per
core); the Cayley structure is hardcoded.

Algorithm (rank-21 factorization on the TensorEngine):
  Cl(3,0) ~= M2(C) via Pauli matrices; the 2x2 complex matmul is done with
  Strassen-7, each complex multiply with the 3-real-mult Gauss trick:
      out = Eo @ ((La @ a) * (Lb @ b))
  with fixed integer matrices La, Lb [21, 8] and Eo [8, 21].  Per
  512-batch supertile: cast-load fp16 position-major tiles, PE-transpose to
  blade-major, PE applies La/Lb (block-diagonal over channels), the
  VectorEngine does the 21-wide elementwise multiply, PE applies Eo.
  The blade-major fp16 result is stored as-is; the host-side gather
  undoes the layout (part of unsharding).  ScalarE does the PSUM->SBUF
  evacuations.  Transposes and matmuls run in separate phases over
  4-supertile blocks so the PE's HAM clock stays at 2.4 GHz during the
  matmul bursts.
"""

import sys

sys.path.insert(0, "/opt/trn_rl_repo")

import numpy as np

import concourse.bass as bass
import concourse.mybir as mybir
from concourse.tile import TileContext
from concourse.bass_utils import run_bass_kernel_spmd


def _patch_wait_spill():
    """The pinned walrus allows at most one sync wait per instruction (two
    for EventSemaphore), but Tile can emit more (e.g. on the kernel-tail
    Drain or on store DMAs).  Post-process the BIR JSON: hoist excess waits
    onto NoOps inserted just before the offending instruction on the same
    engine."""
    import orjson

    if getattr(bass.Bass, "_wait_spill_patch", False):
        return
    orig_to_json_bytes = bass.Bass.to_json_bytes

    def to_json_bytes(self):
        bir = orjson.loads(orig_to_json_bytes(self))
        spill_id = 0
        for fn in bir.get("functions", []):
            for blk in fn.get("blocks", []):
                insts = blk.get("instructions", [])
                out = []
                for ins in insts:
                    si = ins.get("sync_info")
                    cap = 2 if ins.get("opcode") == "EventSemaphore" else 1
                    if si and len(si.get("on_wait", [])) > cap:
                        waits = si["on_wait"]
                        for w in waits[:-cap]:
                            out.append(
                                {
                                    "debug": ins.get("debug", 0),
                                    "engine": ins["engine"],
                                    "ins": [],
                                    "name": f"I-wspill-{spill_id}",
                                    "opcode": "NoOp",
                                    "outs": [],
                                    "text_hint": "wait_spill",
                                    "sync_info": {"on_update": [], "on_wait": [w]},
                                }
                            )
                            spill_id += 1
                        si["on_wait"] = waits[-cap:]
                    out.append(ins)
                blk["instructions"] = out
        return orjson.dumps(bir)

    bass.Bass.to_json_bytes = to_json_bytes
    bass.Bass._wait_spill_patch = True


_patch_wait_spill()

N_CORES = 8
BATCH = 65536
CH = 64
NB = 8
B_CORE = BATCH // N_CORES          # 8192 batches per core
F = CH * NB                        # 512 free elements per batch row
P = 128                            # partitions per tile

R = 21                             # bilinear rank of the factorization
ST = 512                           # batches per supertile
N_ST = B_CORE // ST                # 16
SUBS = [0, 1, 2, 3]                # 4-channel subgroups at 32-aligned bases


def _construct_cayley(metric=(1, 1, 1)):
    d = len(metric)
    n = 1 << d
    bitmaps = sorted(range(n), key=lambda bm: (bin(bm).count("1"), bm))
    b2i = {bm: i for i, bm in enumerate(bitmaps)}
    cay = np.zeros((n, n, n), dtype=np.float32)
    for ia, abm in enumerate(bitmaps):
        for ib, bbm in enumerate(bitmaps):
            t = abm >> 1
            swaps = 0
            while t:
                swaps += bin(t & bbm).count("1")
                t >>= 1
            sign = -1.0 if (swaps & 1) else 1.0
            meet = abm & bbm
            for i in range(d):
                if meet & (1 << i):
                    sign *= metric[i]
            cay[ia, b2i[abm ^ bbm], ib] = sign
    return cay, np.array(bitmaps)


def _rank21_maps():
    s1 = np.array([[0, 1], [1, 0]], dtype=complex)
    s2 = np.array([[0, -1j], [1j, 0]], dtype=complex)
    s3 = np.array([[1, 0], [0, -1]], dtype=complex)
    pauli = {1: s1, 2: s2, 4: s3}
    bitmaps = [0, 1, 2, 4, 3, 5, 6, 7]

    def blade_mat(bm):
        M = np.eye(2, dtype=complex)
        for b in (1, 2, 4):
            if bm & b:
                M = M @ pauli[b]
        return M

    def mat_to_vec8(M):
        v = []
        for r in range(2):
            for c in range(2):
                v += [M[r, c].real, M[r, c].imag]
        return np.array(v)

    Phi = np.stack([mat_to_vec8(blade_mat(bm)) for bm in bitmaps], axis=1)
    Phi_inv = np.linalg.inv(Phi)
    SA = np.array(
        [[1, 0, 0, 1], [0, 0, 1, 1], [1, 0, 0, 0], [0, 0, 0, 1],
         [1, 1, 0, 0], [-1, 0, 1, 0], [0, 1, 0, -1]], dtype=float)
    SB = np.array(
        [[1, 0, 0, 1], [1, 0, 0, 0], [0, 1, 0, -1], [-1, 0, 1, 0],
         [0, 0, 0, 1], [1, 1, 0, 0], [0, 0, 1, 1]], dtype=float)
    SC = np.array(
        [[1, 0, 0, 1, -1, 0, 1], [0, 0, 1, 0, 1, 0, 0],
         [0, 1, 0, 1, 0, 0, 0], [1, -1, 1, 0, 0, 1, 0]], dtype=float)
    L1 = np.zeros((21, 8))
    L2 = np.zeros((21, 8))
    E8 = np.zeros((8, 21))
    for p in range(7):
        ar = np.zeros(8); ai = np.zeros(8); br = np.zeros(8); bi = np.zeros(8)
        for k in range(4):
            ar[2 * k] += SA[p, k]; ai[2 * k + 1] += SA[p, k]
            br[2 * k] += SB[p, k]; bi[2 * k + 1] += SB[p, k]
        L1[3 * p] = ar; L1[3 * p + 1] = ai; L1[3 * p + 2] = ar + ai
        L2[3 * p] = br; L2[3 * p + 1] = bi; L2[3 * p + 2] = br + bi
        for q in range(4):
            w = SC[q, p]
            if w:
                E8[2 * q, 3 * p] += w; E8[2 * q, 3 * p + 1] -= w
                E8[2 * q + 1, 3 * p + 2] += w
                E8[2 * q + 1, 3 * p] -= w; E8[2 * q + 1, 3 * p + 1] -= w
    La = L1 @ Phi
    Lb = L2 @ Phi
    Eo = Phi_inv @ E8
    return La, Lb, Eo


def _blkdiag(M, n):
    r, c = M.shape
    out = np.zeros((n * r, n * c), dtype=M.dtype)
    for i in range(n):
        out[i * r : (i + 1) * r, i * c : (i + 1) * c] = M
    return out


def _build_w_const():
    """fp16 [128, 384] constant: identity + weight matrices.

    WA/WB are [32, 84] block-diag(La.T x4) replicated at all four 32-row
    offsets so any 32-aligned base_partition slice works (matmul requires
    lhsT and rhs to share base_partition).  WE is [84, 32] at base 0."""
    La, Lb, Eo = _rank21_maps()
    w = np.zeros((128, 384), dtype=np.float16)
    cols = {}
    off = 0

    def put(name, M):
        nonlocal off
        p, c = M.shape
        w[:p, off : off + c] = M.astype(np.float16)
        cols[name] = (off, p, c)
        off += c

    put("ID", np.eye(128))
    put("WA", np.tile(_blkdiag(La.T, 4), (4, 1)))   # [128, 84]
    put("WB", np.tile(_blkdiag(Lb.T, 4), (4, 1)))   # [128, 84]
    put("WE", _blkdiag(Eo.T, 4))                    # [84, 32]
    assert off <= 384, off
    return w, cols


_W_CONST, _W_COLS = _build_w_const()


def build_program_v2():
    nc = bass.Bass(num_swdge_queues=4)
    f32 = mybir.dt.float32
    f16 = mybir.dt.float16
    a_ext = nc.declare_dram_parameter("a", [B_CORE, CH, NB], f32, isOutput=False)
    b_ext = nc.declare_dram_parameter("b", [B_CORE, CH, NB], f32, isOutput=False)
    w_ext = nc.declare_dram_parameter("w", list(_W_CONST.shape), f16, isOutput=False)
    # blade-major fp16 output; the host gather undoes the layout
    o_ext = nc.declare_dram_parameter("o", [N_ST, 4, P, ST], f16, isOutput=True)

    a_flat = a_ext.rearrange("b c v -> b (c v)")
    b_flat = b_ext.rearrange("b c v -> b (c v)")
    mult = mybir.AluOpType.mult

    with TileContext(nc) as tc:
        with tc.tile_pool(name="const", bufs=1) as cpool:
            W = cpool.tile([128, _W_CONST.shape[1]], f16)
            nc.sync.dma_start(out=W[:], in_=w_ext[:])

            def wslice(name):
                off, p, c = _W_COLS[name]
                return W[:p, off : off + c]

            ID = wslice("ID")

            with (
                tc.tile_pool(name="io", bufs=2) as io,
                tc.tile_pool(name="mid", bufs=2) as mid,
                tc.tile_pool(name="psA", bufs=2, space="PSUM") as psA,
                tc.tile_pool(name="ps2", bufs=3, space="PSUM") as ps2,
                tc.tile_pool(name="ps3", bufs=3, space="PSUM") as ps3,
            ):
                BLK = 4  # supertiles per phase batch (keeps PE warm ~40us)
                for blk in range(N_ST // BLK):
                    sts = range(blk * BLK, (blk + 1) * BLK)
                    As = {}
                    Bs = {}
                    for st in sts:
                        for bc in range(4):
                            rows = slice(st * ST + bc * P, st * ST + (bc + 1) * P)
                            A = io.tile([P, F], f16, tag=f"A{st % BLK}{bc}")
                            Bt = io.tile([P, F], f16, tag=f"B{st % BLK}{bc}")
                            nc.gpsimd.dma_start(out=A[:], in_=a_flat[rows, :])
                            nc.gpsimd.dma_start(out=Bt[:], in_=b_flat[rows, :])
                            As[(st, bc)] = A
                            Bs[(st, bc)] = Bt
                    # phase 1: all transposes for the block
                    ATs = {}
                    BTs = {}
                    for st in sts:
                        for g in range(4):
                            AT_ps = psA.tile([P, ST], f16, tag="TPS")
                            BT_ps = psA.tile([P, ST], f16, tag="TPS")
                            for bc in range(4):
                                csl = slice(g * 128, (g + 1) * 128)
                                bsl = slice(bc * 128, (bc + 1) * 128)
                                nc.tensor.transpose(
                                    AT_ps[:, bsl], As[(st, bc)][:, csl], ID
                                )
                                nc.tensor.transpose(
                                    BT_ps[:, bsl], Bs[(st, bc)][:, csl], ID
                                )
                            AT = mid.tile([P, ST], f16, tag=f"AT{st % BLK}{g}")
                            BT = mid.tile([P, ST], f16, tag=f"BT{st % BLK}{g}")
                            nc.vector.tensor_copy(out=AT[:], in_=AT_ps[:])
                            nc.vector.tensor_copy(out=BT[:], in_=BT_ps[:])
                            ATs[(st, g)] = AT
                            BTs[(st, g)] = BT
                    # phase 2: dense matmul burst, software-pipelined across
                    # ALL (st, g, sub) units so the PE never stalls at group
                    # boundaries waiting for the ScalarE->VectorE chain
                    M = R * 4  # 84
                    units = [
                        (st, g, sub)
                        for st in sts
                        for g in range(4)
                        for sub in SUBS
                    ]
                    DEPTH = 3
                    uas = {}
                    ubs = {}
                    oTps = {}

                    def emit_pair(u):
                        st, g, sub = u
                        base = 32 * sub
                        rsl = slice(base, base + 32)
                        tp_row = (base, 0) if base >= 96 else None
                        ua_ps = ps2.tile([128, ST], f32, tag="uaps")
                        ub_ps = ps3.tile([128, ST], f32, tag="ubps")
                        nc.tensor.matmul(
                            ua_ps[:M, :], wslice("WA")[rsl, :],
                            ATs[(st, g)][rsl, :],
                            start=True, stop=True, tile_position=tp_row,
                        )
                        nc.tensor.matmul(
                            ub_ps[:M, :], wslice("WB")[rsl, :],
                            BTs[(st, g)][rsl, :],
                            start=True, stop=True, tile_position=tp_row,
                        )
                        uas[u] = ua_ps
                        ubs[u] = ub_ps

                    def consume(u):
                        st, g, sub = u
                        base = 32 * sub
                        rsl = slice(base, base + 32)
                        if (st, g) not in oTps:
                            outT_ps = psA.tile([P, ST], f32, tag="TPS")
                            oTps[(st, g)] = outT_ps
                        outT_ps = oTps[(st, g)]
                        ua = mid.tile([128, ST], f16, tag="ua")
                        nc.scalar.copy(out=ua[:M, :], in_=uas.pop(u)[:M, :])
                        m = mid.tile([128, ST], f16, tag="m")
                        nc.vector.tensor_tensor(
                            out=m[:M, :], in0=ua[:M, :], in1=ubs.pop(u)[:M, :],
                            op=mult,
                        )
                        tp_col = (0, base) if base >= 96 else None
                        nc.tensor.matmul(
                            outT_ps[rsl, :], wslice("WE"), m[:M, :],
                            start=True, stop=True, tile_position=tp_col,
                        )
                        if sub == SUBS[-1]:
                            OT = mid.tile([P, ST], f16, tag=f"OT{st % BLK}{g}")
                            nc.scalar.copy(out=OT[:], in_=outT_ps[:])
                            nc.sync.dma_start(out=o_ext[st, g], in_=OT[:])
                            del oTps[(st, g)]

                    for idx, u in enumerate(units):
                        if idx >= DEPTH:
                            consume(units[idx - DEPTH])
                        emit_pair(u)
                    for u in units[-DEPTH:]:
                        consume(u)
    return nc


def _unshard_core(arr):
    """[N_ST, 4, P, ST] fp16 blade-major -> [B_CORE, CH, NB] f32.

    arr[st, g, c*8+j, t] = out[st*ST + t, 16*g + c, j]"""
    x = np.asarray(arr).reshape(N_ST, 4, 16, NB, ST)
    x = x.transpose(0, 4, 1, 2, 3)           # [st, t, g, c, j]
    return np.ascontiguousarray(x.reshape(B_CORE, CH, NB)).astype(np.float32)


_NC = None
_USES_W = True


def _get_nc():
    global _NC
    if _NC is None:
        _NC = build_program_v2()
    return _NC


LAST_RESULTS = None


def kernel(a, b, cayley=None, **_ignored):
    a = np.ascontiguousarray(np.asarray(a, dtype=np.float32))
    b = np.ascontiguousarray(np.asarray(b, dtype=np.float32))
    assert a.shape == (BATCH, CH, NB) and b.shape == (BATCH, CH, NB)
    nc = _get_nc()
    core_ids = list(range(N_CORES))
    in_maps = []
    for i in core_ids:
        m = {
            "a": a[i * B_CORE : (i + 1) * B_CORE],
            "b": b[i * B_CORE : (i + 1) * B_CORE],
        }
        if _USES_W:
            m["w"] = _W_CONST
        in_maps.append(m)
    res = run_bass_kernel_spmd(nc, in_maps, core_ids)
    global LAST_RESULTS
    LAST_RESULTS = res
    out = np.concatenate(
        [_unshard_core(res.results[i]["o"]) for i in core_ids], axis=0
    )
    return out


# revision 26
# speedup vs baseline: 1.2533x; 1.2533x over previous
"""Clifford algebra geometric product kernel for 8 Trainium2 NeuronCores.

out[..., j] = sum_{i,k} a[..., i] * cayley[i, j, k] * b[..., k]
with cayley the Cl(3,0) (metric [1,1,1]) geometric-product table in
short-lex blade order.  a, b: [65536, 64, 8] float32.

Sharding: pure data parallel over the leading batch axis (8192 batches per
core); the Cayley structure is hardcoded.

Algorithm (rank-21 factorization on the TensorEngine):
  Cl(3,0) ~= M2(C) via Pauli matrices; the 2x2 complex matmul is done with
  Strassen-7, each complex multiply with the 3-real-mult Gauss trick:
      out = Eo @ ((La @ a) * (Lb @ b))
  with fixed integer matrices La, Lb [21, 8] and Eo [8, 21].  Per
  512-batch supertile: cast-load fp16 position-major tiles, PE-transpose to
  blade-major, PE applies La/Lb (block-diagonal over channels), the
  VectorEngine does the 21-wide elementwise multiply, PE applies Eo.
  The blade-major fp16 result is stored as-is; the host-side gather
  undoes the layout (part of unsharding).  ScalarE does the PSUM->SBUF
  evacuations.  Transposes and matmuls run in separate phases over
  4-supertile blocks so the PE's HAM clock stays at 2.4 GHz during the
  matmul bursts.
"""

import sys

sys.path.insert(0, "/opt/trn_rl_repo")

import numpy as np

import concourse.bass as bass
import concourse.mybir as mybir
from concourse.tile import TileContext
from concourse.bass_utils import run_bass_kernel_spmd


def _patch_wait_spill():
    """The pinned walrus allows at most one sync wait per instruction (two
    for EventSemaphore), but Tile can emit more (e.g. on the kernel-tail
    Drain or on store DMAs).  Post-process the BIR JSON: hoist excess waits
    onto NoOps inserted just before the offending instruction on the same
    engine."""
    import orjson

    if getattr(bass.Bass, "_wait_spill_patch", False):
        return
    orig_to_json_bytes = bass.Bass.to_json_bytes

    def to_json_bytes(self):
        bir = orjson.loads(orig_to_json_bytes(self))
        spill_id = 0
        for fn in bir.get("functions", []):
            for blk in fn.get("blocks", []):
                insts = blk.get("instructions", [])
                out = []
                for ins in insts:
                    si = ins.get("sync_info")
                    cap = 2 if ins.get("opcode") == "EventSemaphore" else 1
                    if si and len(si.get("on_wait", [])) > cap:
                        waits = si["on_wait"]
                        for w in waits[:-cap]:
                            out.append(
                                {
                                    "debug": ins.get("debug", 0),
                                    "engine": ins["engine"],
                                    "ins": [],
                                    "name": f"I-wspill-{spill_id}",
                                    "opcode": "NoOp",
                                    "outs": [],
                                    "text_hint": "wait_spill",
                                    "sync_info": {"on_update": [], "on_wait": [w]},
                                }
                            )
                            spill_id += 1
                        si["on_wait"] = waits[-cap:]
                    out.append(ins)
                blk["instructions"] = out
        return orjson.dumps(bir)

    bass.Bass.to_json_bytes = to_json_bytes
    bass.Bass._wait_spill_patch = True


_patch_wait_spill()

N_CORES = 8
BATCH = 65536
CH = 64
NB = 8
B_CORE = BATCH // N_CORES          # 8192 batches per core
F = CH * NB                        # 512 free elements per batch row
P = 128                            # partitions per tile

R = 21                             # bilinear rank of the factorization
ST = 512                           # batches per supertile
N_ST = B_CORE // ST                # 16
SUBS = [0, 1, 2, 3]                # 4-channel subgroups at 32-aligned bases


def _construct_cayley(metric=(1, 1, 1)):
    d = len(metric)
    n = 1 << d
    bitmaps = sorted(range(n), key=lambda bm: (bin(bm).count("1"), bm))
    b2i = {bm: i for i, bm in enumerate(bitmaps)}
    cay = np.zeros((n, n, n), dtype=np.float32)
    for ia, abm in enumerate(bitmaps):
        for ib, bbm in enumerate(bitmaps):
            t = abm >> 1
            swaps = 0
            while t:
                swaps += bin(t & bbm).count("1")
                t >>= 1
            sign = -1.0 if (swaps & 1) else 1.0
            meet = abm & bbm
            for i in range(d):
                if meet & (1 << i):
                    sign *= metric[i]
            cay[ia, b2i[abm ^ bbm], ib] = sign
    return cay, np.array(bitmaps)


def _rank21_maps():
    s1 = np.array([[0, 1], [1, 0]], dtype=complex)
    s2 = np.array([[0, -1j], [1j, 0]], dtype=complex)
    s3 = np.array([[1, 0], [0, -1]], dtype=complex)
    pauli = {1: s1, 2: s2, 4: s3}
    bitmaps = [0, 1, 2, 4, 3, 5, 6, 7]

    def blade_mat(bm):
        M = np.eye(2, dtype=complex)
        for b in (1, 2, 4):
            if bm & b:
                M = M @ pauli[b]
        return M

    def mat_to_vec8(M):
        v = []
        for r in range(2):
            for c in range(2):
                v += [M[r, c].real, M[r, c].imag]
        return np.array(v)

    Phi = np.stack([mat_to_vec8(blade_mat(bm)) for bm in bitmaps], axis=1)
    Phi_inv = np.linalg.inv(Phi)
    SA = np.array(
        [[1, 0, 0, 1], [0, 0, 1, 1], [1, 0, 0, 0], [0, 0, 0, 1],
         [1, 1, 0, 0], [-1, 0, 1, 0], [0, 1, 0, -1]], dtype=float)
    SB = np.array(
        [[1, 0, 0, 1], [1, 0, 0, 0], [0, 1, 0, -1], [-1, 0, 1, 0],
         [0, 0, 0, 1], [1, 1, 0, 0], [0, 0, 1, 1]], dtype=float)
    SC = np.array(
        [[1, 0, 0, 1, -1, 0, 1], [0, 0, 1, 0, 1, 0, 0],
         [0, 1, 0, 1, 0, 0, 0], [1, -1, 1, 0, 0, 1, 0]], dtype=float)
    L1 = np.zeros((21, 8))
    L2 = np.zeros((21, 8))
    E8 = np.zeros((8, 21))
    for p in range(7):
        ar = np.zeros(8); ai = np.zeros(8); br = np.zeros(8); bi = np.zeros(8)
        for k in range(4):
            ar[2 * k] += SA[p, k]; ai[2 * k + 1] += SA[p, k]
            br[2 * k] += SB[p, k]; bi[2 * k + 1] += SB[p, k]
        L1[3 * p] = ar; L1[3 * p + 1] = ai; L1[3 * p + 2] = ar + ai
        L2[3 * p] = br; L2[3 * p + 1] = bi; L2[3 * p + 2] = br + bi
        for q in range(4):
            w = SC[q, p]
            if w:
                E8[2 * q, 3 * p] += w; E8[2 * q, 3 * p + 1] -= w
                E8[2 * q + 1, 3 * p + 2] += w
                E8[2 * q + 1, 3 * p] -= w; E8[2 * q + 1, 3 * p + 1] -= w
    La = L1 @ Phi
    Lb = L2 @ Phi
    Eo = Phi_inv @ E8
    return La, Lb, Eo


def _blkdiag(M, n):
    r, c = M.shape
    out = np.zeros((n * r, n * c), dtype=M.dtype)
    for i in range(n):
        out[i * r : (i + 1) * r, i * c : (i + 1) * c] = M
    return out


def _build_w_const():
    """fp16 [128, 384] constant: identity + weight matrices.

    WA/WB are [32, 84] block-diag(La.T x4) replicated at all four 32-row
    offsets so any 32-aligned base_partition slice works (matmul requires
    lhsT and rhs to share base_partition).  WE is [84, 32] at base 0."""
    La, Lb, Eo = _rank21_maps()
    w = np.zeros((128, 384), dtype=np.float16)
    cols = {}
    off = 0

    def put(name, M):
        nonlocal off
        p, c = M.shape
        w[:p, off : off + c] = M.astype(np.float16)
        cols[name] = (off, p, c)
        off += c

    put("ID", np.eye(128))
    put("WA", np.tile(_blkdiag(La.T, 4), (4, 1)))   # [128, 84]
    put("WB", np.tile(_blkdiag(Lb.T, 4), (4, 1)))   # [128, 84]
    put("WE", _blkdiag(Eo.T, 4))                    # [84, 32]
    assert off <= 384, off
    return w, cols


_W_CONST, _W_COLS = _build_w_const()


def build_program_v2():
    nc = bass.Bass(num_swdge_queues=4)
    f32 = mybir.dt.float32
    f16 = mybir.dt.float16
    a_ext = nc.declare_dram_parameter("a", [B_CORE, CH, NB], f32, isOutput=False)
    b_ext = nc.declare_dram_parameter("b", [B_CORE, CH, NB], f32, isOutput=False)
    w_ext = nc.declare_dram_parameter("w", list(_W_CONST.shape), f16, isOutput=False)
    # blade-major fp16 output; the host gather undoes the layout
    o_ext = nc.declare_dram_parameter("o", [N_ST, 4, P, ST], f16, isOutput=True)

    a_flat = a_ext.rearrange("b c v -> b (c v)")
    b_flat = b_ext.rearrange("b c v -> b (c v)")
    mult = mybir.AluOpType.mult

    with TileContext(nc) as tc:
        with tc.tile_pool(name="const", bufs=1) as cpool:
            W = cpool.tile([128, _W_CONST.shape[1]], f16)
            nc.sync.dma_start(out=W[:], in_=w_ext[:])

            def wslice(name):
                off, p, c = _W_COLS[name]
                return W[:p, off : off + c]

            ID = wslice("ID")

            with (
                tc.tile_pool(name="io", bufs=2) as io,
                tc.tile_pool(name="mid", bufs=2) as mid,
                tc.tile_pool(name="psA", bufs=2, space="PSUM") as psA,
                tc.tile_pool(name="ps2", bufs=3, space="PSUM") as ps2,
                tc.tile_pool(name="ps3", bufs=3, space="PSUM") as ps3,
            ):
                BLK = 4  # supertiles per phase batch (keeps PE warm ~40us)
                for blk in range(N_ST // BLK):
                    sts = range(blk * BLK, (blk + 1) * BLK)
                    As = {}
                    Bs = {}
                    for st in sts:
                        for bc in range(4):
                            rows = slice(st * ST + bc * P, st * ST + (bc + 1) * P)
                            A = io.tile([P, F], f16, tag=f"A{st % BLK}{bc}")
                            Bt = io.tile([P, F], f16, tag=f"B{st % BLK}{bc}")
                            nc.gpsimd.dma_start(out=A[:], in_=a_flat[rows, :])
                            nc.gpsimd.dma_start(out=Bt[:], in_=b_flat[rows, :])
                            As[(st, bc)] = A
                            Bs[(st, bc)] = Bt
                    # phase 1: all transposes for the block
                    ATs = {}
                    BTs = {}
                    for st in sts:
                        for g in range(4):
                            AT_ps = psA.tile([P, ST], f16, tag="TPS")
                            BT_ps = psA.tile([P, ST], f16, tag="TPS")
                            for bc in range(4):
                                csl = slice(g * 128, (g + 1) * 128)
                                bsl = slice(bc * 128, (bc + 1) * 128)
                                nc.tensor.transpose(
                                    AT_ps[:, bsl], As[(st, bc)][:, csl], ID
                                )
                                nc.tensor.transpose(
                                    BT_ps[:, bsl], Bs[(st, bc)][:, csl], ID
                                )
                            AT = mid.tile([P, ST], f16, tag=f"AT{st % BLK}{g}")
                            BT = mid.tile([P, ST], f16, tag=f"BT{st % BLK}{g}")
                            nc.vector.tensor_copy(out=AT[:], in_=AT_ps[:])
                            nc.vector.tensor_copy(out=BT[:], in_=BT_ps[:])
                            ATs[(st, g)] = AT
                            BTs[(st, g)] = BT
                    # phase 2: dense matmul burst, software-pipelined so the
                    # PE issues 6 independent matmuls back-to-back before the
                    # first Eo-matmul that depends on the DVE multiply
                    M = R * 4  # 84
                    for st in sts:
                        for g in range(4):
                            AT = ATs[(st, g)]
                            BT = BTs[(st, g)]
                            outT_ps = psA.tile([P, ST], f32, tag="TPS")

                            uas = {}
                            ubs = {}

                            def emit_pair(sub, AT=AT, BT=BT, uas=uas, ubs=ubs):
                                base = 32 * sub
                                rsl = slice(base, base + 32)
                                tp_row = (base, 0) if base >= 96 else None
                                ua_ps = ps2.tile([128, ST], f32, tag="uaps")
                                ub_ps = ps3.tile([128, ST], f32, tag="ubps")
                                nc.tensor.matmul(
                                    ua_ps[:M, :], wslice("WA")[rsl, :], AT[rsl, :],
                                    start=True, stop=True, tile_position=tp_row,
                                )
                                nc.tensor.matmul(
                                    ub_ps[:M, :], wslice("WB")[rsl, :], BT[rsl, :],
                                    start=True, stop=True, tile_position=tp_row,
                                )
                                uas[sub] = ua_ps
                                ubs[sub] = ub_ps

                            for sub in (0, 1, 2):
                                emit_pair(sub)
                            for sub in SUBS:
                                base = 32 * sub
                                rsl = slice(base, base + 32)
                                ua = mid.tile([128, ST], f16, tag="ua")
                                nc.scalar.copy(out=ua[:M, :], in_=uas[sub][:M, :])
                                m = mid.tile([128, ST], f16, tag="m")
                                nc.vector.tensor_tensor(
                                    out=m[:M, :], in0=ua[:M, :],
                                    in1=ubs[sub][:M, :], op=mult,
                                )
                                if sub + 3 in SUBS:
                                    emit_pair(sub + 3)
                                tp_col = (0, base) if base >= 96 else None
                                nc.tensor.matmul(
                                    outT_ps[rsl, :], wslice("WE"), m[:M, :],
                                    start=True, stop=True, tile_position=tp_col,
                                )
                            OT = mid.tile([P, ST], f16, tag=f"OT{st % BLK}{g}")
                            nc.scalar.copy(out=OT[:], in_=outT_ps[:])
                            nc.sync.dma_start(out=o_ext[st, g], in_=OT[:])
    return nc


def _unshard_core(arr):
    """[N_ST, 4, P, ST] fp16 blade-major -> [B_CORE, CH, NB] f32.

    arr[st, g, c*8+j, t] = out[st*ST + t, 16*g + c, j]"""
    x = np.asarray(arr).reshape(N_ST, 4, 16, NB, ST)
    x = x.transpose(0, 4, 1, 2, 3)           # [st, t, g, c, j]
    return np.ascontiguousarray(x.reshape(B_CORE, CH, NB)).astype(np.float32)


_NC = None
_USES_W = True


def _get_nc():
    global _NC
    if _NC is None:
        _NC = build_program_v2()
    return _NC


LAST_RESULTS = None


def kernel(a, b, cayley=None, **_ignored):
    a = np.ascontiguousarray(np.asarray(a, dtype=np.float32))
    b = np.ascontiguousarray(np.asarray(b, dtype=np.float32))
    assert a.shape == (BATCH, CH, NB) and b.shape == (BATCH, CH, NB)
    nc = _get_nc()
    core_ids = list(range(N_CORES))
    in_maps = []
    for i in core_ids:
        m = {
            "a": a[i * B_CORE : (i + 1) * B_CORE],
            "b": b[i * B_CORE : (i + 1) * B_CORE],
        }
        if _USES_W:
            m["w"] = _W_CONST
        in_maps.append(m)
    res = run_bass_kernel_spmd(nc, in_maps, core_ids)
    global LAST_RESULTS
    LAST_RESULTS = res
    out = np.concatenate(
        [_unshard_core(res.results[i]["o"]) for i in core_ids], axis=0
    )
    return out
